# revision 19
# baseline (speedup 1.0000x reference)
# CATS-SwiGLU decode kernel for TRN2 (8 NeuronCores, SPMD tensor-parallel).
# v7: mixed-precision weight streaming, dual-engine consumption.
#   gate fp16 (flag flips near |x1|>thr are the error wall), up/down fp8-e3m4
#   scaled by 128 (cancelled via the mask constant 1/2^14).
# Measured engine rates: PE GEMV (moving weights) ~N/2GHz+120ns per matmul
# (~170 Gelem/s), DVE affine_mul_reduce ~121 Gelem/s.  Neither alone can keep
# up with the ~378 GB/s HWDGE stream, so each matrix is split: f-chunks
# [0:640) reduce on the DVE over f-major [128,4096] tiles (accumulator lands
# f-on-partitions, pre-transposed for z), f-rows [640:1376) on the PE as
# d-major GEMVs; the down projection splits d-cols 2432 (PE) / 1664 (DVE).
# All gate/up/x DMA rides the sync HWDGE ring in consumption order (SWDGE is
# ~3x slower at descriptor generation - never use gpsimd rings for bulk);
# the down stream rides the scalar ring, pinned behind the last gate piece
# by the dummy-DMA WAW trick.
import sys

for _p in ("/opt/trn_rl_repo",):
    if _p not in sys.path:
        sys.path.insert(0, _p)

import numpy as np
import ml_dtypes

import concourse.bass as bass
import concourse.tile as tile
from concourse import bacc, mybir
from concourse.bass_utils import run_bass_kernel_spmd
from concourse.masks import make_identity

D = 4096
FF = 11008
NCORES = 8
FSH = FF // NCORES            # 1376 f-rows per core
NCD = D // 128                # 32 d-chunks (contraction for gate/up)
NCF = (FSH + 127) // 128      # 11 f-chunks
LASTF = FSH - 128 * (NCF - 1)  # 96 rows in the last f chunk

FDC = 5                       # f-chunks on the DVE for gate/up
FD = FDC * 128                # 640
FP = FSH - FD                 # 736 PE-part f width
FT = ((0, 512), (512, FP - 512))
FPAD = NCF * 128              # 1408 padded f-rows for wdp

DP = 2432                     # d-cols on PE for down
NDVG = (D - DP) // 128        # 13 DVE down groups
DT = ((0, 512), (512, 512), (1024, 512), (1536, 512), (2048, 384))
ZS = ((0, 512, 0, 4), (512, 512, 4, 4), (1024, 352, 8, 3))  # z_row slices

WSCL = 128.0                  # e3m4 scale; 1/WSCL^2 folded into the mask
TPC = (1, 2, 2)               # T (DVE-layout) piece sizes in chunks
PPC = (8, 8, 8, 8)            # P (PE-layout) piece sizes in d-chunks
GBUFS = 3                     # gate P-piece pool depth
WDDPC = (4, 4, 3, 2)          # wdd pieces, in d-groups
WDPPC = (3, 3, 3, 2)          # wdp pieces, in f-chunks

F32 = mybir.dt.float32
F16 = mybir.dt.float16
FP8 = mybir.dt.float8e3
NF16 = np.float16
NF8 = ml_dtypes.float8_e3m4

_CACHE = {}


def _bcast(ap, parts):
    return bass.AP(tensor=ap.tensor, offset=ap.offset, ap=[[0, parts]] + list(ap.ap))


def _build_nc():
    nc = bacc.Bacc("TRN2", target_bir_lowering=False, debug=False)

    xf_d = nc.dram_tensor("xf", [128, D + NCD], F16, kind="ExternalInput")
    wut_d = nc.dram_tensor("wut", [128, FDC * D], FP8, kind="ExternalInput")
    wgt_d = nc.dram_tensor("wgt", [128, FDC * D], F16, kind="ExternalInput")
    wup_d = nc.dram_tensor("wup", [128, NCD * FP], FP8, kind="ExternalInput")
    wgp_d = nc.dram_tensor("wgp", [128, NCD * FP], F16, kind="ExternalInput")
    wdp_d = nc.dram_tensor("wdp", [128, NCF * DP], FP8, kind="ExternalInput")
    wdd_d = nc.dram_tensor("wdd", [128, NDVG * FSH], FP8, kind="ExternalInput")
    thr_d = nc.dram_tensor("thr", [1], F32, kind="ExternalInput")
    outp_d = nc.dram_tensor("outp", [DP], F32, kind="ExternalOutput")
    outd_d = nc.dram_tensor("outd", [128, NDVG], F32, kind="ExternalOutput")

    with tile.TileContext(nc) as tc:
        with (
            tc.tile_pool(name="const", bufs=1) as cp,
            tc.tile_pool(name="gpool", bufs=GBUFS) as gpool,
            tc.tile_pool(name="acts", bufs=1) as acts,
        ):
            # x (replicated row + column-chunked) is the first sync transfer;
            # thr rides the scalar ring.
            xf = cp.tile([128, D + NCD], F16)
            nc.sync.dma_start(out=xf[:], in_=xf_d.ap())
            xrep = xf[:, 0:D]
            xcol = xf[:, D : D + NCD]
            thr_sb = cp.tile([128, 1], F32)
            nc.scalar.dma_start(out=thr_sb[:], in_=_bcast(thr_d.ap(), 128))
            ones_f = cp.tile([1, 1], F16)
            nc.vector.memset(ones_f[:], 1.0)
            ones_c = cp.tile([1, 128], F16)
            nc.vector.memset(ones_c[:], 1.0)
            ident = cp.tile([128, 128], F16)
            make_identity(nc, ident[:])

            # ACT warmups: preload the Silu/Abs tables before the hot path.
            warm = acts.tile([128, 1], F32)
            nc.scalar.activation(
                warm[:], thr_sb[:], mybir.ActivationFunctionType.Silu
            )
            nc.scalar.activation(
                warm[:], thr_sb[:], mybir.ActivationFunctionType.Abs
            )
            nc.scalar.copy(warm[:], thr_sb[:])

            # resident weight tiles (fp8 streams are small enough to hold)
            wut_sb = acts.tile([128, FDC * D], FP8)
            wgt_sb = acts.tile([128, FDC * D], F16)
            wup_sb = acts.tile([128, NCD * FP], FP8)
            wdp_sb = acts.tile([128, NCF * DP], FP8)
            wdd_sb = acts.tile([128, NDVG * FSH], FP8)

            # activation scratch
            accg = acts.tile([128, FDC], F32)   # DVE-part gate accum
            accu = acts.tile([128, FDC], F32)   # DVE-part up accum
            dve_scr = acts.tile([128, D], F16)
            g_row = acts.tile([1, FP], F16)
            u_row = acts.tile([1, FP], F16)
            trig8 = acts.tile([1, 8], FP8)
            tg_col = acts.tile([128, NCF], F32)
            x1c = acts.tile([128, NCF], F32)
            abc = acts.tile([128, NCF], F32)
            mkc = acts.tile([128, NCF], F32)
            xmc = acts.tile([128, NCF], F32)
            u_col = acts.tile([128, NCF], F32)
            z_col = acts.tile([128, NCF], F16)
            z_row = acts.tile([1, FSH], F16)
            zrep = acts.tile([128, FSH], F16)
            osb = acts.tile([1, DP], F32)
            outd_sb = acts.tile([128, NDVG], F32)

            # ---- gate/up stream: all on the sync HWDGE ring, interleaved in
            # consumption order ----
            # (kind, matrix, offset_chunks, nchunks); matrix 0=up, 1=gate
            order = []
            toff_ = [0, 0]
            poff_ = [0, 0]
            for i, npc in enumerate(TPC):
                for mi in (0, 1):
                    order.append(("T", mi, toff_[mi], npc))
                    toff_[mi] += npc
                for mi in (0, 1):
                    order.append(("P", mi, poff_[mi], PPC[i]))
                    poff_[mi] += PPC[i]
            for mi in (0, 1):
                order.append(("P", mi, poff_[mi], PPC[3]))
                poff_[mi] += PPC[3]

            gtiles = {}
            for kind, mi, off, npc in order:
                if kind == "T":
                    dram = wut_d if mi == 0 else wgt_d
                    sb = wut_sb if mi == 0 else wgt_sb
                    sl = slice(off * D, (off + npc) * D)
                    nc.sync.dma_start(out=sb[:, sl], in_=dram.ap()[:, sl])
                elif mi == 0:
                    sl = slice(off * FP, (off + npc) * FP)
                    nc.sync.dma_start(out=wup_sb[:, sl], in_=wup_d.ap()[:, sl])
                else:
                    t = gpool.tile([128, 8 * FP], F16, tag="gw", name="gw")
                    sl = slice(off * FP, (off + npc) * FP)
                    nc.sync.dma_start(
                        out=t[:, 0 : npc * FP], in_=wgp_d.ap()[:, sl]
                    )
                    gtiles[(mi, off)] = t

            with tc.tile_pool(name="ps1", bufs=1, space="PSUM") as ps1:
                pup = ps1.tile([1, 1024], F32)
                pgate = ps1.tile([1, 1024], F32)
                pzcu = ps1.tile([128, 16], F32)
                pzcg = ps1.tile([128, 16], F32)

                # consumption, in stream order: DVE affines for T pieces, PE
                # GEMVs for P pieces
                for kind, mi, off, npc in order:
                    if kind == "T":
                        sb = wut_sb if mi == 0 else wgt_sb
                        acct = accu if mi == 0 else accg
                        for j in range(npc):
                            c = off + j
                            nc.vector.affine_mul_reduce(
                                out=dve_scr[:, 0:D],
                                accum_out=acct[:, c : c + 1],
                                in0=sb[:, c * D : (c + 1) * D],
                                in1=xrep[:],
                                scale=1.0,
                                bias=0.0,
                            )
                    else:
                        accp = pup if mi == 0 else pgate
                        for cc in range(npc):
                            c = off + cc
                            if mi == 0:
                                rh = wup_sb[:, c * FP : (c + 1) * FP]
                            else:
                                t = gtiles[(mi, off)]
                                rh = t[:, cc * FP : (cc + 1) * FP]
                            for toff, tlen in FT:
                                nc.tensor.matmul(
                                    out=accp[0:1, toff : toff + tlen],
                                    lhsT=xcol[:, c : c + 1],
                                    rhs=rh[:, toff : toff + tlen],
                                    start=(c == 0),
                                    stop=(c == NCD - 1),
                                )

                # ---- down-stream DMAs, pinned behind the last gate P piece
                trig = gtiles[(1, NCD - PPC[3])]
                nc.scalar.copy(trig8[0:1, 0:8], trig[0:1, 0:8])
                nc.scalar.dma_start(
                    out=wdd_sb[0:1, 0 : 3 * 4 * FSH + 1 : 4 * FSH],
                    in_=trig8[0:1, 0:4],
                )
                nc.scalar.dma_start(
                    out=wdp_sb[0:1, 0 : 3 * 3 * DP + 1 : 3 * DP],
                    in_=trig8[0:1, 4:8],
                )
                do = 0
                for npc in WDDPC:
                    w = npc * FSH
                    nc.scalar.dma_start(
                        out=wdd_sb[:, do : do + w],
                        in_=wdd_d.ap()[:, do : do + w],
                    )
                    do += w
                po = 0
                for npc in WDPPC:
                    w = npc * DP
                    nc.scalar.dma_start(
                        out=wdp_sb[:, po : po + w],
                        in_=wdp_d.ap()[:, po : po + w],
                    )
                    po += w

                # ---- u/g rows to column form; unified elementwise chain ----
                nc.scalar.copy(u_row[0:1, 0:FP], pup[0:1, 0:FP])
                nc.vector.tensor_copy(g_row[0:1, 0:FP], pgate[0:1, 0:FP])
                for c in range(NCF - FDC):
                    pc = 128 if FDC + c < NCF - 1 else LASTF
                    nc.tensor.matmul(
                        out=pzcu[0:pc, c : c + 1],
                        lhsT=u_row[0:1, c * 128 : c * 128 + pc],
                        rhs=ones_f[0:1, 0:1],
                        start=True,
                        stop=True,
                    )
                    nc.tensor.matmul(
                        out=pzcg[0:pc, c : c + 1],
                        lhsT=g_row[0:1, c * 128 : c * 128 + pc],
                        rhs=ones_f[0:1, 0:1],
                        start=True,
                        stop=True,
                    )
                nc.scalar.copy(u_col[:, 0:FDC], accu[:, 0:FDC])
                nc.scalar.copy(u_col[:, FDC:NCF], pzcu[:, 0 : NCF - FDC])
                nc.scalar.copy(tg_col[:, 0:FDC], accg[:, 0:FDC])
                nc.scalar.copy(tg_col[:, FDC:NCF], pzcg[:, 0 : NCF - FDC])
                nc.scalar.activation(
                    x1c[:, 0:NCF], tg_col[:, 0:NCF],
                    mybir.ActivationFunctionType.Silu,
                )
                nc.scalar.activation(
                    abc[:, 0:NCF], x1c[:, 0:NCF],
                    mybir.ActivationFunctionType.Abs,
                )
                # mask carries the 1/WSCL^2 rescale of pup*(scaled wd)
                nc.vector.tensor_scalar(
                    out=mkc[:, 0:NCF], in0=abc[:, 0:NCF],
                    scalar1=thr_sb[:], scalar2=1.0 / (WSCL * WSCL),
                    op0=mybir.AluOpType.is_gt,
                    op1=mybir.AluOpType.mult,
                )
                nc.vector.tensor_mul(xmc[:, 0:NCF], x1c[:, 0:NCF], mkc[:, 0:NCF])
                nc.vector.tensor_mul(z_col[:, 0:NCF], u_col[:, 0:NCF], xmc[:, 0:NCF])

            with tc.tile_pool(name="ps2", bufs=1, space="PSUM") as ps2:
                pzrow = ps2.tile([1, 512], F32)
                przep = ps2.tile([128, 512], F32)
                pdp = ps2.tile([1, DP], F32)

                # z_row + zrep, slice-pipelined through one psum bank pair
                for soff, slen, c0, ncc in ZS:
                    for i in range(ncc):
                        c = c0 + i
                        pc = 128 if c < NCF - 1 else LASTF
                        nc.tensor.matmul(
                            out=pzrow[0:1, i * 128 : i * 128 + pc],
                            lhsT=z_col[0:pc, c : c + 1],
                            rhs=ident[0:pc, 0:pc],
                            start=True,
                            stop=True,
                        )
                    nc.scalar.copy(
                        z_row[0:1, soff : soff + slen], pzrow[0:1, 0:slen]
                    )
                    nc.tensor.matmul(
                        out=przep[:, 0:slen],
                        lhsT=ones_c[0:1, 0:128],
                        rhs=z_row[0:1, soff : soff + slen],
                        start=True,
                        stop=True,
                    )
                    nc.vector.tensor_copy(
                        zrep[:, soff : soff + slen], przep[:, 0:slen]
                    )

                # PE down part; copy out tile-wise as the last chunk retires
                for c in range(NCF):
                    pc = 128 if c < NCF - 1 else LASTF
                    last = c == NCF - 1
                    for ti, (toff, tlen) in enumerate(DT):
                        nc.tensor.matmul(
                            out=pdp[0:1, toff : toff + tlen],
                            lhsT=z_col[0:pc, c : c + 1],
                            rhs=wdp_sb[0:pc, c * DP + toff : c * DP + toff + tlen],
                            start=(c == 0),
                            stop=last,
                        )
                        if last:
                            sl = slice(toff, toff + tlen)
                            if ti % 2 == 0:
                                nc.scalar.copy(osb[0:1, sl], pdp[0:1, sl])
                            else:
                                nc.vector.tensor_copy(osb[0:1, sl], pdp[0:1, sl])

                # DVE down part
                for g in range(NDVG):
                    nc.vector.affine_mul_reduce(
                        out=dve_scr[:, 0:FSH],
                        accum_out=outd_sb[:, g : g + 1],
                        in0=wdd_sb[:, g * FSH : (g + 1) * FSH],
                        in1=zrep[:],
                        scale=1.0,
                        bias=0.0,
                    )

            nc.sync.dma_start(out=outp_d.ap(), in_=osb[:])
            nc.sync.dma_start(out=outd_d.ap(), in_=outd_sb[:])

    nc.compile()
    return nc


def _get_nc():
    if "nc" not in _CACHE:
        _CACHE["nc"] = _build_nc()
    return _CACHE["nc"]


def _q8(W):
    return np.clip(
        np.asarray(W, dtype=np.float32) * WSCL, -15.5, 15.5
    ).astype(NF8)


def make_in_maps(x, Wup, Wgatet, Wdownt, threshold):
    """Shard full inputs into the 8 per-core input maps."""
    x_flat = np.asarray(x, dtype=np.float32).reshape(D)
    xcol = np.ascontiguousarray(x_flat.reshape(NCD, 128).T).astype(NF16)
    xf = np.ascontiguousarray(
        np.concatenate(
            [np.broadcast_to(x_flat.astype(NF16), (128, D)), xcol], axis=1
        )
    )
    thr = np.asarray(threshold, dtype=np.float32).reshape(1)
    Wup = np.asarray(Wup, dtype=np.float32)
    Wgatet = np.asarray(Wgatet, dtype=np.float32)
    Wdownt = np.asarray(Wdownt, dtype=np.float32)
    in_maps = []
    for i in range(NCORES):
        sl = slice(i * FSH, (i + 1) * FSH)
        wg_slice = Wgatet[:, sl]                  # [D, FSH] d-major
        wu_slice = Wup[sl, :]                     # [FSH, D] f-major
        wd_slice = Wdownt[sl, :]                  # [FSH, D] f-major

        # DVE layouts: f-major [128f, D] per chunk for f-rows [0:FD)
        wgT = np.ascontiguousarray(wg_slice.T)    # [FSH, D] f-major
        wgt = (
            wgT[:FD].reshape(FDC, 128, D).transpose(1, 0, 2)
            .reshape(128, FDC * D).astype(NF16)
        )
        wut = _q8(
            wu_slice[:FD].reshape(FDC, 128, D).transpose(1, 0, 2)
            .reshape(128, FDC * D)
        )
        # PE layouts: d-major [128d, FP] per chunk for f-rows [FD:FSH)
        wgp = (
            wg_slice[:, FD:].reshape(NCD, 128, FP).transpose(1, 0, 2)
            .reshape(128, NCD * FP).astype(NF16)
        )
        wuT = np.ascontiguousarray(wu_slice.T)    # [D, FSH] d-major
        wup = _q8(
            wuT[:, FD:].reshape(NCD, 128, FP).transpose(1, 0, 2)
            .reshape(128, NCD * FP)
        )
        wd_pad = np.zeros((FPAD, DP), dtype=np.float32)
        wd_pad[:FSH] = wd_slice[:, :DP]
        wdp = _q8(
            wd_pad.reshape(NCF, 128, DP).transpose(1, 0, 2).reshape(128, NCF * DP)
        )
        wdT = np.ascontiguousarray(wd_slice[:, DP:].T)  # [D-DP, FSH] d-major
        wdd = _q8(
            wdT.reshape(NDVG, 128, FSH).transpose(1, 0, 2).reshape(128, NDVG * FSH)
        )
        in_maps.append(
            {
                "xf": xf,
                "wut": np.ascontiguousarray(wut),
                "wgt": np.ascontiguousarray(wgt),
                "wup": np.ascontiguousarray(wup),
                "wgp": np.ascontiguousarray(wgp),
                "wdp": np.ascontiguousarray(wdp),
                "wdd": np.ascontiguousarray(wdd),
                "thr": thr,
            }
        )
    return in_maps


def run_sharded(x, Wup, Wgatet, Wdownt, threshold, trace=False, tmpdir=None):
    """Run on the 8 NeuronCores; returns (full_output, BassKernelResults)."""
    nc = _get_nc()
    in_maps = make_in_maps(x, Wup, Wgatet, Wdownt, threshold)
    res = run_bass_kernel_spmd(
        nc, in_maps, list(range(NCORES)), trace=trace, tmpdir=tmpdir
    )
    acc = np.zeros(D, dtype=np.float64)
    for r in res.results:
        acc[:DP] += r["outp"].reshape(DP).astype(np.float64)
        acc[DP:] += r["outd"].T.reshape(D - DP).astype(np.float64)
    out = acc.astype(np.float32).reshape(1, 1, D)
    return out, res


def kernel(x, Wup, Wgatet, Wdownt, threshold):
    out, _ = run_sharded(x, Wup, Wgatet, Wdownt, threshold)
    return out


# revision 21
# speedup vs baseline: 1.1065x; 1.1065x over previous
# CATS-SwiGLU decode kernel for TRN2 (8 NeuronCores, SPMD tensor-parallel).
# v7: mixed-precision weight streaming, dual-engine consumption.
#   gate fp16 (flag flips near |x1|>thr are the error wall), up/down fp8-e3m4
#   scaled by 128 (cancelled via the mask constant 1/2^14).
# Measured engine rates: PE GEMV (moving weights) ~N/2GHz+120ns per matmul
# (~170 Gelem/s), DVE affine_mul_reduce ~121 Gelem/s.  Neither alone can keep
# up with the ~378 GB/s HWDGE stream, so each matrix is split: f-chunks
# [0:640) reduce on the DVE over f-major [128,4096] tiles (accumulator lands
# f-on-partitions, pre-transposed for z), f-rows [640:1376) on the PE as
# d-major GEMVs; the down projection splits d-cols 2432 (PE) / 1664 (DVE).
# All gate/up/x DMA rides the sync HWDGE ring in consumption order (SWDGE is
# ~3x slower at descriptor generation - never use gpsimd rings for bulk);
# the down stream rides the scalar ring, pinned behind the last gate piece
# by the dummy-DMA WAW trick.
import sys

for _p in ("/opt/trn_rl_repo",):
    if _p not in sys.path:
        sys.path.insert(0, _p)

import numpy as np
import ml_dtypes

import concourse.bass as bass
import concourse.tile as tile
from concourse import bacc, mybir
from concourse.bass_utils import run_bass_kernel_spmd
from concourse.masks import make_identity

D = 4096
FF = 11008
NCORES = 8
FSH = FF // NCORES            # 1376 f-rows per core
NCD = D // 128                # 32 d-chunks (contraction for gate/up)
NCF = (FSH + 127) // 128      # 11 f-chunks
LASTF = FSH - 128 * (NCF - 1)  # 96 rows in the last f chunk

FDC = 4                       # f-chunks on the DVE for gate/up
FD = FDC * 128                # 640
FP = FSH - FD                 # 736 PE-part f width
FT = ((0, 512), (512, FP - 512))  # (512, 352)
FPAD = NCF * 128              # 1408 padded f-rows for wdp

DP = 2304                     # d-cols on PE for down
NDVG = (D - DP) // 128        # 14 DVE down groups
DT = ((0, 512), (512, 512), (1024, 512), (1536, 512), (2048, 256))
ZS = ((0, 512, 0, 4), (512, 512, 4, 4), (1024, 352, 8, 3))  # z_row slices

WSCL = 128.0                  # e3m4 scale; 1/WSCL^2 folded into the mask
TPC = (1, 2, 1)               # T (DVE-layout) piece sizes in chunks
PPC = (8, 8, 8, 8)            # P (PE-layout) piece sizes in d-chunks
GBUFS = 3                     # gate P-piece pool depth
WDDPC = (4, 4, 4, 2)          # wdd pieces, in d-groups
WDPPC = (3, 3, 3, 2)          # wdp pieces, in f-chunks

F32 = mybir.dt.float32
F16 = mybir.dt.float16
FP8 = mybir.dt.float8e3
NF16 = np.float16
NF8 = ml_dtypes.float8_e3m4

_CACHE = {}


def _bcast(ap, parts):
    return bass.AP(tensor=ap.tensor, offset=ap.offset, ap=[[0, parts]] + list(ap.ap))


def _build_nc():
    nc = bacc.Bacc("TRN2", target_bir_lowering=False, debug=False)

    xf_d = nc.dram_tensor("xf", [128, D + NCD], F16, kind="ExternalInput")
    wut_d = nc.dram_tensor("wut", [128, FDC * D], FP8, kind="ExternalInput")
    wgt_d = nc.dram_tensor("wgt", [128, FDC * D], F16, kind="ExternalInput")
    wup_d = nc.dram_tensor("wup", [128, NCD * FP], FP8, kind="ExternalInput")
    wgp_d = nc.dram_tensor("wgp", [128, NCD * FP], F16, kind="ExternalInput")
    wdp_d = nc.dram_tensor("wdp", [128, NCF * DP], FP8, kind="ExternalInput")
    wdd_d = nc.dram_tensor("wdd", [128, NDVG * FSH], FP8, kind="ExternalInput")
    thr_d = nc.dram_tensor("thr", [1], F32, kind="ExternalInput")
    outp_d = nc.dram_tensor("outp", [DP], F32, kind="ExternalOutput")
    outd_d = nc.dram_tensor("outd", [128, NDVG], F32, kind="ExternalOutput")

    with tile.TileContext(nc) as tc:
        with (
            tc.tile_pool(name="const", bufs=1) as cp,
            tc.tile_pool(name="gpool", bufs=GBUFS) as gpool,
            tc.tile_pool(name="acts", bufs=1) as acts,
        ):
            # x (replicated row + column-chunked) is the first sync transfer;
            # thr rides the scalar ring.
            xf = cp.tile([128, D + NCD], F16)
            nc.sync.dma_start(out=xf[:], in_=xf_d.ap())
            xrep = xf[:, 0:D]
            xcol = xf[:, D : D + NCD]
            thr_sb = cp.tile([128, 1], F32)
            nc.scalar.dma_start(out=thr_sb[:], in_=_bcast(thr_d.ap(), 128))
            ones_f = cp.tile([1, 1], F16)
            nc.vector.memset(ones_f[:], 1.0)
            ones_c = cp.tile([1, 128], F16)
            nc.vector.memset(ones_c[:], 1.0)
            ident = cp.tile([128, 128], F16)
            make_identity(nc, ident[:])

            # ACT warmups: preload the Silu/Abs tables before the hot path.
            warm = acts.tile([128, 1], F32)
            nc.scalar.activation(
                warm[:], thr_sb[:], mybir.ActivationFunctionType.Silu
            )
            nc.scalar.activation(
                warm[:], thr_sb[:], mybir.ActivationFunctionType.Abs
            )
            nc.scalar.copy(warm[:], thr_sb[:])

            # resident weight tiles (fp8 streams are small enough to hold)
            wut_sb = acts.tile([128, FDC * D], FP8)
            wgt_sb = acts.tile([128, FDC * D], F16)
            wup_sb = acts.tile([128, NCD * FP], FP8)
            wdp_sb = acts.tile([128, NCF * DP], FP8)
            wdd_sb = acts.tile([128, NDVG * FSH], FP8)

            # activation scratch
            accg = acts.tile([128, FDC], F32)   # DVE-part gate accum
            accu = acts.tile([128, FDC], F32)   # DVE-part up accum
            dve_scr = acts.tile([128, D], F16)
            g_row = acts.tile([1, FP], F16)
            u_row = acts.tile([1, FP], F16)
            trig8 = acts.tile([1, 8], FP8)
            tg_col = acts.tile([128, NCF], F32)
            x1c = acts.tile([128, NCF], F32)
            abc = acts.tile([128, NCF], F32)
            mkc = acts.tile([128, NCF], F32)
            xmc = acts.tile([128, NCF], F32)
            u_col = acts.tile([128, NCF], F32)
            z_col = acts.tile([128, NCF], F16)
            z_row = acts.tile([1, FSH], F16)
            zrep = acts.tile([128, FSH], F16)
            osb = acts.tile([1, DP], F32)
            outd_sb = acts.tile([128, NDVG], F32)

            # ---- gate/up stream: all on the sync HWDGE ring, interleaved in
            # consumption order ----
            # (kind, matrix, offset_chunks, nchunks); matrix 0=up, 1=gate
            order = []
            toff_ = [0, 0]
            poff_ = [0, 0]
            for i, npc in enumerate(TPC):
                for mi in (0, 1):
                    order.append(("T", mi, toff_[mi], npc))
                    toff_[mi] += npc
                    order.append(("P", mi, poff_[mi], PPC[i]))
                    poff_[mi] += PPC[i]
            for mi in (0, 1):
                order.append(("P", mi, poff_[mi], PPC[3]))
                poff_[mi] += PPC[3]

            gtiles = {}
            for kind, mi, off, npc in order:
                if kind == "T":
                    dram = wut_d if mi == 0 else wgt_d
                    sb = wut_sb if mi == 0 else wgt_sb
                    sl = slice(off * D, (off + npc) * D)
                    nc.sync.dma_start(out=sb[:, sl], in_=dram.ap()[:, sl])
                elif mi == 0:
                    sl = slice(off * FP, (off + npc) * FP)
                    nc.sync.dma_start(out=wup_sb[:, sl], in_=wup_d.ap()[:, sl])
                else:
                    t = gpool.tile([128, 8 * FP], F16, tag="gw", name="gw")
                    sl = slice(off * FP, (off + npc) * FP)
                    nc.sync.dma_start(
                        out=t[:, 0 : npc * FP], in_=wgp_d.ap()[:, sl]
                    )
                    gtiles[(mi, off)] = t

            with tc.tile_pool(name="ps1", bufs=1, space="PSUM") as ps1:
                pup = ps1.tile([1, 1024], F32)
                pgate = ps1.tile([1, 1024], F32)
                pzcu = ps1.tile([128, 16], F32)
                pzcg = ps1.tile([128, 16], F32)

                # consumption, in stream order: DVE affines for T pieces, PE
                # GEMVs for P pieces
                for kind, mi, off, npc in order:
                    if kind == "T":
                        sb = wut_sb if mi == 0 else wgt_sb
                        acct = accu if mi == 0 else accg
                        for j in range(npc):
                            c = off + j
                            nc.vector.affine_mul_reduce(
                                out=dve_scr[:, 0:D],
                                accum_out=acct[:, c : c + 1],
                                in0=sb[:, c * D : (c + 1) * D],
                                in1=xrep[:],
                                scale=1.0,
                                bias=0.0,
                            )
                    else:
                        accp = pup if mi == 0 else pgate
                        for cc in range(npc):
                            c = off + cc
                            if mi == 0:
                                rh = wup_sb[:, c * FP : (c + 1) * FP]
                            else:
                                t = gtiles[(mi, off)]
                                rh = t[:, cc * FP : (cc + 1) * FP]
                            for toff, tlen in FT:
                                nc.tensor.matmul(
                                    out=accp[0:1, toff : toff + tlen],
                                    lhsT=xcol[:, c : c + 1],
                                    rhs=rh[:, toff : toff + tlen],
                                    start=(c == 0),
                                    stop=(c == NCD - 1),
                                )

                # ---- down-stream DMAs, pinned behind the last gate P piece
                trig = gtiles[(1, NCD - PPC[3])]
                nc.scalar.copy(trig8[0:1, 0:8], trig[0:1, 0:8])
                nc.scalar.dma_start(
                    out=wdd_sb[0:1, 0 : 3 * 4 * FSH + 1 : 4 * FSH],
                    in_=trig8[0:1, 0:4],
                )
                nc.scalar.dma_start(
                    out=wdp_sb[0:1, 0 : 3 * 3 * DP + 1 : 3 * DP],
                    in_=trig8[0:1, 4:8],
                )
                do = 0
                for npc in WDDPC:
                    w = npc * FSH
                    nc.scalar.dma_start(
                        out=wdd_sb[:, do : do + w],
                        in_=wdd_d.ap()[:, do : do + w],
                    )
                    do += w
                po = 0
                for npc in WDPPC:
                    w = npc * DP
                    nc.scalar.dma_start(
                        out=wdp_sb[:, po : po + w],
                        in_=wdp_d.ap()[:, po : po + w],
                    )
                    po += w

                # ---- u/g rows to column form; unified elementwise chain ----
                nc.scalar.copy(u_row[0:1, 0:FP], pup[0:1, 0:FP])
                nc.scalar.copy(g_row[0:1, 0:FP], pgate[0:1, 0:FP])
                for c in range(NCF - FDC):
                    pc = 128 if FDC + c < NCF - 1 else LASTF
                    nc.tensor.matmul(
                        out=pzcu[0:pc, c : c + 1],
                        lhsT=u_row[0:1, c * 128 : c * 128 + pc],
                        rhs=ones_f[0:1, 0:1],
                        start=True,
                        stop=True,
                    )
                    nc.tensor.matmul(
                        out=pzcg[0:pc, c : c + 1],
                        lhsT=g_row[0:1, c * 128 : c * 128 + pc],
                        rhs=ones_f[0:1, 0:1],
                        start=True,
                        stop=True,
                    )
                nc.scalar.copy(u_col[:, 0:FDC], accu[:, 0:FDC])
                nc.scalar.copy(u_col[:, FDC:NCF], pzcu[:, 0 : NCF - FDC])
                nc.scalar.copy(tg_col[:, 0:FDC], accg[:, 0:FDC])
                nc.scalar.copy(tg_col[:, FDC:NCF], pzcg[:, 0 : NCF - FDC])
                nc.scalar.activation(
                    x1c[:, 0:NCF], tg_col[:, 0:NCF],
                    mybir.ActivationFunctionType.Silu,
                )
                nc.scalar.activation(
                    abc[:, 0:NCF], x1c[:, 0:NCF],
                    mybir.ActivationFunctionType.Abs,
                )
                # mask carries the 1/WSCL^2 rescale of pup*(scaled wd)
                nc.vector.tensor_scalar(
                    out=mkc[:, 0:NCF], in0=abc[:, 0:NCF],
                    scalar1=thr_sb[:], scalar2=1.0 / (WSCL * WSCL),
                    op0=mybir.AluOpType.is_gt,
                    op1=mybir.AluOpType.mult,
                )
                nc.vector.tensor_mul(xmc[:, 0:NCF], x1c[:, 0:NCF], mkc[:, 0:NCF])
                nc.vector.tensor_mul(z_col[:, 0:NCF], u_col[:, 0:NCF], xmc[:, 0:NCF])

            with tc.tile_pool(name="ps2", bufs=1, space="PSUM") as ps2:
                pzrow = ps2.tile([1, 512], F32)
                przep = ps2.tile([128, 512], F32)
                pdp = ps2.tile([1, DP], F32)

                # z_row + zrep, slice-pipelined through one psum bank pair
                for soff, slen, c0, ncc in ZS:
                    for i in range(ncc):
                        c = c0 + i
                        pc = 128 if c < NCF - 1 else LASTF
                        nc.tensor.matmul(
                            out=pzrow[0:1, i * 128 : i * 128 + pc],
                            lhsT=z_col[0:pc, c : c + 1],
                            rhs=ident[0:pc, 0:pc],
                            start=True,
                            stop=True,
                        )
                    nc.scalar.copy(
                        z_row[0:1, soff : soff + slen], pzrow[0:1, 0:slen]
                    )
                    nc.tensor.matmul(
                        out=przep[:, 0:slen],
                        lhsT=ones_c[0:1, 0:128],
                        rhs=z_row[0:1, soff : soff + slen],
                        start=True,
                        stop=True,
                    )
                    nc.vector.tensor_copy(
                        zrep[:, soff : soff + slen], przep[:, 0:slen]
                    )

                # PE down part; copy out tile-wise as the last chunk retires
                for c in range(NCF):
                    pc = 128 if c < NCF - 1 else LASTF
                    last = c == NCF - 1
                    for ti, (toff, tlen) in enumerate(DT):
                        nc.tensor.matmul(
                            out=pdp[0:1, toff : toff + tlen],
                            lhsT=z_col[0:pc, c : c + 1],
                            rhs=wdp_sb[0:pc, c * DP + toff : c * DP + toff + tlen],
                            start=(c == 0),
                            stop=last,
                        )
                        if last:
                            sl = slice(toff, toff + tlen)
                            if ti % 2 == 0:
                                nc.scalar.copy(osb[0:1, sl], pdp[0:1, sl])
                            else:
                                nc.vector.tensor_copy(osb[0:1, sl], pdp[0:1, sl])

                # DVE down part
                for g in range(NDVG):
                    nc.vector.affine_mul_reduce(
                        out=dve_scr[:, 0:FSH],
                        accum_out=outd_sb[:, g : g + 1],
                        in0=wdd_sb[:, g * FSH : (g + 1) * FSH],
                        in1=zrep[:],
                        scale=1.0,
                        bias=0.0,
                    )

            nc.sync.dma_start(out=outp_d.ap(), in_=osb[:])
            nc.sync.dma_start(out=outd_d.ap(), in_=outd_sb[:])

    nc.compile()
    return nc


def _get_nc():
    if "nc" not in _CACHE:
        _CACHE["nc"] = _build_nc()
    return _CACHE["nc"]


def _q8(W):
    return np.clip(
        np.asarray(W, dtype=np.float32) * WSCL, -15.5, 15.5
    ).astype(NF8)


def make_in_maps(x, Wup, Wgatet, Wdownt, threshold):
    """Shard full inputs into the 8 per-core input maps."""
    x_flat = np.asarray(x, dtype=np.float32).reshape(D)
    xcol = np.ascontiguousarray(x_flat.reshape(NCD, 128).T).astype(NF16)
    xf = np.ascontiguousarray(
        np.concatenate(
            [np.broadcast_to(x_flat.astype(NF16), (128, D)), xcol], axis=1
        )
    )
    thr = np.asarray(threshold, dtype=np.float32).reshape(1)
    Wup = np.asarray(Wup, dtype=np.float32)
    Wgatet = np.asarray(Wgatet, dtype=np.float32)
    Wdownt = np.asarray(Wdownt, dtype=np.float32)
    in_maps = []
    for i in range(NCORES):
        sl = slice(i * FSH, (i + 1) * FSH)
        wg_slice = Wgatet[:, sl]                  # [D, FSH] d-major
        wu_slice = Wup[sl, :]                     # [FSH, D] f-major
        wd_slice = Wdownt[sl, :]                  # [FSH, D] f-major

        # DVE layouts: f-major [128f, D] per chunk for f-rows [0:FD)
        wgT = np.ascontiguousarray(wg_slice.T)    # [FSH, D] f-major
        wgt = (
            wgT[:FD].reshape(FDC, 128, D).transpose(1, 0, 2)
            .reshape(128, FDC * D).astype(NF16)
        )
        wut = _q8(
            wu_slice[:FD].reshape(FDC, 128, D).transpose(1, 0, 2)
            .reshape(128, FDC * D)
        )
        # PE layouts: d-major [128d, FP] per chunk for f-rows [FD:FSH)
        wgp = (
            wg_slice[:, FD:].reshape(NCD, 128, FP).transpose(1, 0, 2)
            .reshape(128, NCD * FP).astype(NF16)
        )
        wuT = np.ascontiguousarray(wu_slice.T)    # [D, FSH] d-major
        wup = _q8(
            wuT[:, FD:].reshape(NCD, 128, FP).transpose(1, 0, 2)
            .reshape(128, NCD * FP)
        )
        wd_pad = np.zeros((FPAD, DP), dtype=np.float32)
        wd_pad[:FSH] = wd_slice[:, :DP]
        wdp = _q8(
            wd_pad.reshape(NCF, 128, DP).transpose(1, 0, 2).reshape(128, NCF * DP)
        )
        wdT = np.ascontiguousarray(wd_slice[:, DP:].T)  # [D-DP, FSH] d-major
        wdd = _q8(
            wdT.reshape(NDVG, 128, FSH).transpose(1, 0, 2).reshape(128, NDVG * FSH)
        )
        in_maps.append(
            {
                "xf": xf,
                "wut": np.ascontiguousarray(wut),
                "wgt": np.ascontiguousarray(wgt),
                "wup": np.ascontiguousarray(wup),
                "wgp": np.ascontiguousarray(wgp),
                "wdp": np.ascontiguousarray(wdp),
                "wdd": np.ascontiguousarray(wdd),
                "thr": thr,
            }
        )
    return in_maps


def run_sharded(x, Wup, Wgatet, Wdownt, threshold, trace=False, tmpdir=None):
    """Run on the 8 NeuronCores; returns (full_output, BassKernelResults)."""
    nc = _get_nc()
    in_maps = make_in_maps(x, Wup, Wgatet, Wdownt, threshold)
    res = run_bass_kernel_spmd(
        nc, in_maps, list(range(NCORES)), trace=trace, tmpdir=tmpdir
    )
    acc = np.zeros(D, dtype=np.float64)
    for r in res.results:
        acc[:DP] += r["outp"].reshape(DP).astype(np.float64)
        acc[DP:] += r["outd"].T.reshape(D - DP).astype(np.float64)
    out = acc.astype(np.float32).reshape(1, 1, D)
    return out, res


def kernel(x, Wup, Wgatet, Wdownt, threshold):
    out, _ = run_sharded(x, Wup, Wgatet, Wdownt, threshold)
    return out


# revision 22
# speedup vs baseline: 1.1244x; 1.0162x over previous
# CATS-SwiGLU decode kernel for TRN2 (8 NeuronCores, SPMD tensor-parallel).
# v7: mixed-precision weight streaming, dual-engine consumption.
#   gate fp16 (flag flips near |x1|>thr are the error wall), up/down fp8-e3m4
#   scaled by 128 (cancelled via the mask constant 1/2^14).
# Measured engine rates: PE GEMV (moving weights) ~N/2GHz+120ns per matmul
# (~170 Gelem/s), DVE affine_mul_reduce ~121 Gelem/s.  Neither alone can keep
# up with the ~378 GB/s HWDGE stream, so each matrix is split: f-chunks
# [0:640) reduce on the DVE over f-major [128,4096] tiles (accumulator lands
# f-on-partitions, pre-transposed for z), f-rows [640:1376) on the PE as
# d-major GEMVs; the down projection splits d-cols 2432 (PE) / 1664 (DVE).
# All gate/up/x DMA rides the sync HWDGE ring in consumption order (SWDGE is
# ~3x slower at descriptor generation - never use gpsimd rings for bulk);
# the down stream rides the scalar ring, pinned behind the last gate piece
# by the dummy-DMA WAW trick.
import sys

for _p in ("/opt/trn_rl_repo",):
    if _p not in sys.path:
        sys.path.insert(0, _p)

import numpy as np
import ml_dtypes

import concourse.bass as bass
import concourse.tile as tile
from concourse import bacc, mybir
from concourse.bass_utils import run_bass_kernel_spmd
from concourse.masks import make_identity

D = 4096
FF = 11008
NCORES = 8
FSH = FF // NCORES            # 1376 f-rows per core
NCD = D // 128                # 32 d-chunks (contraction for gate/up)
NCF = (FSH + 127) // 128      # 11 f-chunks
LASTF = FSH - 128 * (NCF - 1)  # 96 rows in the last f chunk

FDC = 4                       # f-chunks on the DVE for gate/up
FD = FDC * 128                # 640
FP = FSH - FD                 # 736 PE-part f width
FT = ((0, 512), (512, FP - 512))  # (512, 352)
FPAD = NCF * 128              # 1408 padded f-rows for wdp

DP = 2304                     # d-cols on PE for down
NDVG = (D - DP) // 128        # 14 DVE down groups
DT = ((0, 512), (512, 512), (1024, 512), (1536, 512), (2048, 256))
ZS = ((0, 512, 0, 4), (512, 512, 4, 4), (1024, 352, 8, 3))  # z_row slices

WSCL = 128.0                  # e3m4 scale; 1/WSCL^2 folded into the mask
TPC = (1, 2, 1)               # T (DVE-layout) piece sizes in chunks
PPC = (8, 8, 8, 8)            # P (PE-layout) piece sizes in d-chunks
GBUFS = 3                     # gate P-piece pool depth
WDDPC = (4, 4, 4, 2)          # wdd pieces, in d-groups
WDPPC = (3, 3, 3, 2)          # wdp pieces, in f-chunks

F32 = mybir.dt.float32
F16 = mybir.dt.float16
FP8 = mybir.dt.float8e3
NF16 = np.float16
NF8 = ml_dtypes.float8_e3m4

_CACHE = {}


def _bcast(ap, parts):
    return bass.AP(tensor=ap.tensor, offset=ap.offset, ap=[[0, parts]] + list(ap.ap))


def _build_nc():
    nc = bacc.Bacc("TRN2", target_bir_lowering=False, debug=False)

    xf_d = nc.dram_tensor("xf", [128, D + NCD], F16, kind="ExternalInput")
    wut_d = nc.dram_tensor("wut", [128, FDC * D], FP8, kind="ExternalInput")
    wgt_d = nc.dram_tensor("wgt", [128, FDC * D], F16, kind="ExternalInput")
    wup_d = nc.dram_tensor("wup", [128, NCD * FP], FP8, kind="ExternalInput")
    wgp_d = nc.dram_tensor("wgp", [128, NCD * FP], F16, kind="ExternalInput")
    wdp_d = nc.dram_tensor("wdp", [128, NCF * DP], FP8, kind="ExternalInput")
    wdd_d = nc.dram_tensor("wdd", [128, NDVG * FSH], FP8, kind="ExternalInput")
    thr_d = nc.dram_tensor("thr", [1], F32, kind="ExternalInput")
    outp_d = nc.dram_tensor("outp", [DP], F32, kind="ExternalOutput")
    outd_d = nc.dram_tensor("outd", [128, NDVG], F32, kind="ExternalOutput")

    with tile.TileContext(nc) as tc:
        with (
            tc.tile_pool(name="const", bufs=1) as cp,
            tc.tile_pool(name="gpool", bufs=GBUFS) as gpool,
            tc.tile_pool(name="acts", bufs=1) as acts,
        ):
            # x (replicated row + column-chunked) is the first sync transfer;
            # thr rides the scalar ring.
            xf = cp.tile([128, D + NCD], F16)
            nc.sync.dma_start(out=xf[:], in_=xf_d.ap())
            xrep = xf[:, 0:D]
            xcol = xf[:, D : D + NCD]
            thr_sb = cp.tile([128, 1], F32)
            nc.scalar.dma_start(out=thr_sb[:], in_=_bcast(thr_d.ap(), 128))
            ones_f = cp.tile([1, 1], F16)
            nc.vector.memset(ones_f[:], 1.0)
            ones_c = cp.tile([1, 128], F16)
            nc.vector.memset(ones_c[:], 1.0)
            ident = cp.tile([128, 128], F16)
            make_identity(nc, ident[:])

            # ACT warmups: preload the Silu/Abs tables before the hot path.
            warm = acts.tile([128, 1], F32)
            nc.scalar.activation(
                warm[:], thr_sb[:], mybir.ActivationFunctionType.Silu
            )
            nc.scalar.activation(
                warm[:], thr_sb[:], mybir.ActivationFunctionType.Abs
            )
            nc.scalar.copy(warm[:], thr_sb[:])

            # resident weight tiles (fp8 streams are small enough to hold)
            wut_sb = acts.tile([128, FDC * D], FP8)
            wgt_sb = acts.tile([128, FDC * D], F16)
            wup_sb = acts.tile([128, NCD * FP], FP8)
            wdp_sb = acts.tile([128, NCF * DP], FP8)
            wdd_sb = acts.tile([128, NDVG * FSH], FP8)

            # activation scratch
            accg = acts.tile([128, FDC], F32)   # DVE-part gate accum
            accu = acts.tile([128, FDC], F32)   # DVE-part up accum
            dve_scr = acts.tile([128, D], F16)
            g_row = acts.tile([1, FP], F16)
            u_row = acts.tile([1, FP], F16)
            trig8 = acts.tile([1, 8], FP8)
            tg_col = acts.tile([128, NCF], F32)
            x1c = acts.tile([128, NCF], F32)
            abc = acts.tile([128, NCF], F32)
            mkc = acts.tile([128, NCF], F32)
            xmc = acts.tile([128, NCF], F32)
            u_col = acts.tile([128, NCF], F32)
            z_col = acts.tile([128, NCF], F16)
            z_row = acts.tile([1, FSH], F16)
            zrep = acts.tile([128, FSH], F16)
            osb = acts.tile([1, DP], F32)
            outd_sb = acts.tile([128, NDVG], F32)

            # ---- gate/up stream: all on the sync HWDGE ring, interleaved in
            # consumption order ----
            # (kind, matrix, offset_chunks, nchunks); matrix 0=up, 1=gate
            order = []
            toff_ = [0, 0]
            poff_ = [0, 0]
            for i, npc in enumerate(TPC):
                for mi in (0, 1):
                    order.append(("T", mi, toff_[mi], npc))
                    toff_[mi] += npc
                    order.append(("P", mi, poff_[mi], PPC[i]))
                    poff_[mi] += PPC[i]
            for mi in (0, 1):
                order.append(("P", mi, poff_[mi], PPC[3]))
                poff_[mi] += PPC[3]

            gtiles = {}
            for kind, mi, off, npc in order:
                if kind == "T":
                    dram = wut_d if mi == 0 else wgt_d
                    sb = wut_sb if mi == 0 else wgt_sb
                    sl = slice(off * D, (off + npc) * D)
                    nc.sync.dma_start(out=sb[:, sl], in_=dram.ap()[:, sl])
                elif mi == 0:
                    sl = slice(off * FP, (off + npc) * FP)
                    nc.sync.dma_start(out=wup_sb[:, sl], in_=wup_d.ap()[:, sl])
                else:
                    t = gpool.tile([128, 8 * FP], F16, tag="gw", name="gw")
                    sl = slice(off * FP, (off + npc) * FP)
                    nc.sync.dma_start(
                        out=t[:, 0 : npc * FP], in_=wgp_d.ap()[:, sl]
                    )
                    gtiles[(mi, off)] = t

            with tc.tile_pool(name="ps1", bufs=1, space="PSUM") as ps1:
                pup = ps1.tile([1, 1024], F32)
                pgate = ps1.tile([1, 1024], F32)
                pzcu = ps1.tile([128, 16], F32)
                pzcg = ps1.tile([128, 16], F32)

                # consumption, in stream order: DVE affines for T pieces, PE
                # GEMVs for P pieces
                for kind, mi, off, npc in order:
                    if kind == "T":
                        sb = wut_sb if mi == 0 else wgt_sb
                        acct = accu if mi == 0 else accg
                        for j in range(npc):
                            c = off + j
                            nc.vector.affine_mul_reduce(
                                out=dve_scr[:, 0:D],
                                accum_out=acct[:, c : c + 1],
                                in0=sb[:, c * D : (c + 1) * D],
                                in1=xrep[:],
                                scale=1.0,
                                bias=0.0,
                            )
                    else:
                        accp = pup if mi == 0 else pgate
                        for cc in range(npc):
                            c = off + cc
                            if mi == 0:
                                rh = wup_sb[:, c * FP : (c + 1) * FP]
                            else:
                                t = gtiles[(mi, off)]
                                rh = t[:, cc * FP : (cc + 1) * FP]
                            for toff, tlen in FT:
                                nc.tensor.matmul(
                                    out=accp[0:1, toff : toff + tlen],
                                    lhsT=xcol[:, c : c + 1],
                                    rhs=rh[:, toff : toff + tlen],
                                    start=(c == 0),
                                    stop=(c == NCD - 1),
                                )

                # ---- down-stream DMAs, pinned behind the last gate P piece
                trig = gtiles[(1, NCD - PPC[3])]
                nc.scalar.copy(trig8[0:1, 0:8], trig[0:1, 0:8])
                nc.scalar.dma_start(
                    out=wdd_sb[0:1, 0 : 3 * 4 * FSH + 1 : 4 * FSH],
                    in_=trig8[0:1, 0:4],
                )
                nc.scalar.dma_start(
                    out=wdp_sb[0:1, 0 : 3 * 3 * DP + 1 : 3 * DP],
                    in_=trig8[0:1, 4:8],
                )
                do = 0
                for npc in WDDPC:
                    w = npc * FSH
                    nc.scalar.dma_start(
                        out=wdd_sb[:, do : do + w],
                        in_=wdd_d.ap()[:, do : do + w],
                    )
                    do += w
                po = 0
                for npc in WDPPC:
                    w = npc * DP
                    nc.scalar.dma_start(
                        out=wdp_sb[:, po : po + w],
                        in_=wdp_d.ap()[:, po : po + w],
                    )
                    po += w

                # ---- u/g rows to column form; unified elementwise chain ----
                nc.scalar.copy(u_row[0:1, 0:FP], pup[0:1, 0:FP])
                nc.scalar.copy(g_row[0:1, 0:FP], pgate[0:1, 0:FP])
                for c in range(NCF - FDC):
                    pc = 128 if FDC + c < NCF - 1 else LASTF
                    nc.tensor.matmul(
                        out=pzcu[0:pc, c : c + 1],
                        lhsT=u_row[0:1, c * 128 : c * 128 + pc],
                        rhs=ones_f[0:1, 0:1],
                        start=True,
                        stop=True,
                    )
                    nc.tensor.matmul(
                        out=pzcg[0:pc, c : c + 1],
                        lhsT=g_row[0:1, c * 128 : c * 128 + pc],
                        rhs=ones_f[0:1, 0:1],
                        start=True,
                        stop=True,
                    )
                nc.scalar.copy(u_col[:, 0:FDC], accu[:, 0:FDC])
                nc.scalar.copy(u_col[:, FDC:NCF], pzcu[:, 0 : NCF - FDC])
                nc.scalar.copy(tg_col[:, 0:FDC], accg[:, 0:FDC])
                nc.scalar.copy(tg_col[:, FDC:NCF], pzcg[:, 0 : NCF - FDC])
                nc.scalar.activation(
                    x1c[:, 0:NCF], tg_col[:, 0:NCF],
                    mybir.ActivationFunctionType.Silu,
                )
                nc.scalar.activation(
                    abc[:, 0:NCF], x1c[:, 0:NCF],
                    mybir.ActivationFunctionType.Abs,
                )
                # mask carries the 1/WSCL^2 rescale of pup*(scaled wd)
                nc.vector.tensor_scalar(
                    out=mkc[:, 0:NCF], in0=abc[:, 0:NCF],
                    scalar1=thr_sb[:], scalar2=1.0 / (WSCL * WSCL),
                    op0=mybir.AluOpType.is_gt,
                    op1=mybir.AluOpType.mult,
                )
                nc.vector.tensor_mul(xmc[:, 0:NCF], x1c[:, 0:NCF], mkc[:, 0:NCF])
                nc.vector.tensor_mul(z_col[:, 0:NCF], u_col[:, 0:NCF], xmc[:, 0:NCF])

            with tc.tile_pool(name="ps2", bufs=1, space="PSUM") as ps2:
                pzrow = ps2.tile([1, 512], F32)
                przep = ps2.tile([128, 512], F32)
                pdp = ps2.tile([1, DP], F32)

                # z_row + zrep, slice-pipelined through one psum bank pair
                for soff, slen, c0, ncc in ZS:
                    for i in range(ncc):
                        c = c0 + i
                        pc = 128 if c < NCF - 1 else LASTF
                        nc.tensor.matmul(
                            out=pzrow[0:1, i * 128 : i * 128 + pc],
                            lhsT=z_col[0:pc, c : c + 1],
                            rhs=ident[0:pc, 0:pc],
                            start=True,
                            stop=True,
                        )
                    nc.scalar.copy(
                        z_row[0:1, soff : soff + slen], pzrow[0:1, 0:slen]
                    )
                    nc.tensor.matmul(
                        out=przep[:, 0:slen],
                        lhsT=ones_c[0:1, 0:128],
                        rhs=z_row[0:1, soff : soff + slen],
                        start=True,
                        stop=True,
                    )
                    nc.vector.tensor_copy(
                        zrep[:, soff : soff + slen], przep[:, 0:slen]
                    )

                # DVE down part first in program order so its affines are
                # not queued behind the osb copies (which wait on PE-down)
                for g in range(NDVG):
                    nc.vector.affine_mul_reduce(
                        out=dve_scr[:, 0:FSH],
                        accum_out=outd_sb[:, g : g + 1],
                        in0=wdd_sb[:, g * FSH : (g + 1) * FSH],
                        in1=zrep[:],
                        scale=1.0,
                        bias=0.0,
                    )

                # PE down part; ACT copies out tile-wise as chunks retire
                for c in range(NCF):
                    pc = 128 if c < NCF - 1 else LASTF
                    last = c == NCF - 1
                    for ti, (toff, tlen) in enumerate(DT):
                        nc.tensor.matmul(
                            out=pdp[0:1, toff : toff + tlen],
                            lhsT=z_col[0:pc, c : c + 1],
                            rhs=wdp_sb[0:pc, c * DP + toff : c * DP + toff + tlen],
                            start=(c == 0),
                            stop=last,
                        )
                        if last:
                            sl = slice(toff, toff + tlen)
                            nc.scalar.copy(osb[0:1, sl], pdp[0:1, sl])

            nc.sync.dma_start(out=outp_d.ap(), in_=osb[:])
            nc.sync.dma_start(out=outd_d.ap(), in_=outd_sb[:])

    nc.compile()
    return nc


def _get_nc():
    if "nc" not in _CACHE:
        _CACHE["nc"] = _build_nc()
    return _CACHE["nc"]


def _q8(W):
    return np.clip(
        np.asarray(W, dtype=np.float32) * WSCL, -15.5, 15.5
    ).astype(NF8)


def make_in_maps(x, Wup, Wgatet, Wdownt, threshold):
    """Shard full inputs into the 8 per-core input maps."""
    x_flat = np.asarray(x, dtype=np.float32).reshape(D)
    xcol = np.ascontiguousarray(x_flat.reshape(NCD, 128).T).astype(NF16)
    xf = np.ascontiguousarray(
        np.concatenate(
            [np.broadcast_to(x_flat.astype(NF16), (128, D)), xcol], axis=1
        )
    )
    thr = np.asarray(threshold, dtype=np.float32).reshape(1)
    Wup = np.asarray(Wup, dtype=np.float32)
    Wgatet = np.asarray(Wgatet, dtype=np.float32)
    Wdownt = np.asarray(Wdownt, dtype=np.float32)
    in_maps = []
    for i in range(NCORES):
        sl = slice(i * FSH, (i + 1) * FSH)
        wg_slice = Wgatet[:, sl]                  # [D, FSH] d-major
        wu_slice = Wup[sl, :]                     # [FSH, D] f-major
        wd_slice = Wdownt[sl, :]                  # [FSH, D] f-major

        # DVE layouts: f-major [128f, D] per chunk for f-rows [0:FD)
        wgT = np.ascontiguousarray(wg_slice.T)    # [FSH, D] f-major
        wgt = (
            wgT[:FD].reshape(FDC, 128, D).transpose(1, 0, 2)
            .reshape(128, FDC * D).astype(NF16)
        )
        wut = _q8(
            wu_slice[:FD].reshape(FDC, 128, D).transpose(1, 0, 2)
            .reshape(128, FDC * D)
        )
        # PE layouts: d-major [128d, FP] per chunk for f-rows [FD:FSH)
        wgp = (
            wg_slice[:, FD:].reshape(NCD, 128, FP).transpose(1, 0, 2)
            .reshape(128, NCD * FP).astype(NF16)
        )
        wuT = np.ascontiguousarray(wu_slice.T)    # [D, FSH] d-major
        wup = _q8(
            wuT[:, FD:].reshape(NCD, 128, FP).transpose(1, 0, 2)
            .reshape(128, NCD * FP)
        )
        wd_pad = np.zeros((FPAD, DP), dtype=np.float32)
        wd_pad[:FSH] = wd_slice[:, :DP]
        wdp = _q8(
            wd_pad.reshape(NCF, 128, DP).transpose(1, 0, 2).reshape(128, NCF * DP)
        )
        wdT = np.ascontiguousarray(wd_slice[:, DP:].T)  # [D-DP, FSH] d-major
        wdd = _q8(
            wdT.reshape(NDVG, 128, FSH).transpose(1, 0, 2).reshape(128, NDVG * FSH)
        )
        in_maps.append(
            {
                "xf": xf,
                "wut": np.ascontiguousarray(wut),
                "wgt": np.ascontiguousarray(wgt),
                "wup": np.ascontiguousarray(wup),
                "wgp": np.ascontiguousarray(wgp),
                "wdp": np.ascontiguousarray(wdp),
                "wdd": np.ascontiguousarray(wdd),
                "thr": thr,
            }
        )
    return in_maps


def run_sharded(x, Wup, Wgatet, Wdownt, threshold, trace=False, tmpdir=None):
    """Run on the 8 NeuronCores; returns (full_output, BassKernelResults)."""
    nc = _get_nc()
    in_maps = make_in_maps(x, Wup, Wgatet, Wdownt, threshold)
    res = run_bass_kernel_spmd(
        nc, in_maps, list(range(NCORES)), trace=trace, tmpdir=tmpdir
    )
    acc = np.zeros(D, dtype=np.float64)
    for r in res.results:
        acc[:DP] += r["outp"].reshape(DP).astype(np.float64)
        acc[DP:] += r["outd"].T.reshape(D - DP).astype(np.float64)
    out = acc.astype(np.float32).reshape(1, 1, D)
    return out, res


def kernel(x, Wup, Wgatet, Wdownt, threshold):
    out, _ = run_sharded(x, Wup, Wgatet, Wdownt, threshold)
    return out


# revision 23
# speedup vs baseline: 1.1838x; 1.0528x over previous
# CATS-SwiGLU decode kernel for TRN2 (8 NeuronCores, SPMD tensor-parallel).
# v7: mixed-precision weight streaming, dual-engine consumption.
#   gate fp16 (flag flips near |x1|>thr are the error wall), up/down fp8-e3m4
#   scaled by 128 (cancelled via the mask constant 1/2^14).
# Measured engine rates: PE GEMV (moving weights) ~N/2GHz+120ns per matmul
# (~170 Gelem/s), DVE affine_mul_reduce ~121 Gelem/s.  Neither alone can keep
# up with the ~378 GB/s HWDGE stream, so each matrix is split: f-chunks
# [0:640) reduce on the DVE over f-major [128,4096] tiles (accumulator lands
# f-on-partitions, pre-transposed for z), f-rows [640:1376) on the PE as
# d-major GEMVs; the down projection splits d-cols 2432 (PE) / 1664 (DVE).
# All gate/up/x DMA rides the sync HWDGE ring in consumption order (SWDGE is
# ~3x slower at descriptor generation - never use gpsimd rings for bulk);
# the down stream rides the scalar ring, pinned behind the last gate piece
# by the dummy-DMA WAW trick.
import sys

for _p in ("/opt/trn_rl_repo",):
    if _p not in sys.path:
        sys.path.insert(0, _p)

import numpy as np
import ml_dtypes

import concourse.bass as bass
import concourse.tile as tile
from concourse import bacc, mybir
from concourse.bass_utils import run_bass_kernel_spmd
from concourse.masks import make_identity

D = 4096
FF = 11008
NCORES = 8
FSH = FF // NCORES            # 1376 f-rows per core
NCD = D // 128                # 32 d-chunks (contraction for gate/up)
NCF = (FSH + 127) // 128      # 11 f-chunks
LASTF = FSH - 128 * (NCF - 1)  # 96 rows in the last f chunk

FDC = 4                       # f-chunks on the DVE for gate/up
FD = FDC * 128                # 640
FP = FSH - FD                 # 736 PE-part f width
FT = ((0, 512), (512, FP - 512))  # (512, 352)
FPAD = NCF * 128              # 1408 padded f-rows for wdp

DP = 2816                     # d-cols on PE for down
NDVG = (D - DP) // 128        # 10 DVE down groups
DT = ((0, 512), (512, 512), (1024, 512), (1536, 512), (2048, 512), (2560, 256))
ZS = ((0, 512, 0, 4), (512, 512, 4, 4), (1024, 352, 8, 3))  # z_row slices

WSCL = 128.0                  # e3m4 scale; 1/WSCL^2 folded into the mask
TPC = (1, 1, 1, 1)            # T (DVE-layout) piece sizes in chunks
PPC = (8, 8, 8, 8)            # P (PE-layout) piece sizes in d-chunks
GBUFS = 3                     # gate P-piece pool depth
WDDPC = (4, 4, 2)             # wdd pieces, in d-groups
WDPPC = (3, 3, 3, 2)          # wdp pieces, in f-chunks

F32 = mybir.dt.float32
F16 = mybir.dt.float16
FP8 = mybir.dt.float8e3
NF16 = np.float16
NF8 = ml_dtypes.float8_e3m4

_CACHE = {}


def _bcast(ap, parts):
    return bass.AP(tensor=ap.tensor, offset=ap.offset, ap=[[0, parts]] + list(ap.ap))


def _build_nc():
    nc = bacc.Bacc("TRN2", target_bir_lowering=False, debug=False)

    xf_d = nc.dram_tensor("xf", [128, D + NCD], F16, kind="ExternalInput")
    wut_d = nc.dram_tensor("wut", [128, FDC * D], FP8, kind="ExternalInput")
    wgt_d = nc.dram_tensor("wgt", [128, FDC * D], F16, kind="ExternalInput")
    wup_d = nc.dram_tensor("wup", [128, NCD * FP], FP8, kind="ExternalInput")
    wgp_d = nc.dram_tensor("wgp", [128, NCD * FP], F16, kind="ExternalInput")
    wdp_d = nc.dram_tensor("wdp", [128, NCF * DP], FP8, kind="ExternalInput")
    wdd_d = nc.dram_tensor("wdd", [128, NDVG * FSH], FP8, kind="ExternalInput")
    thr_d = nc.dram_tensor("thr", [1], F32, kind="ExternalInput")
    outp_d = nc.dram_tensor("outp", [DP], F32, kind="ExternalOutput")
    outd_d = nc.dram_tensor("outd", [128, NDVG], F32, kind="ExternalOutput")

    with tile.TileContext(nc) as tc:
        with (
            tc.tile_pool(name="const", bufs=1) as cp,
            tc.tile_pool(name="gpool", bufs=GBUFS) as gpool,
            tc.tile_pool(name="acts", bufs=1) as acts,
        ):
            # x (replicated row + column-chunked) is the first sync transfer;
            # thr rides the scalar ring.
            xf = cp.tile([128, D + NCD], F16)
            nc.sync.dma_start(out=xf[:], in_=xf_d.ap())
            xrep = xf[:, 0:D]
            xcol = xf[:, D : D + NCD]
            thr_sb = cp.tile([128, 1], F32)
            nc.scalar.dma_start(out=thr_sb[:], in_=_bcast(thr_d.ap(), 128))
            ones_f = cp.tile([1, 1], F16)
            nc.vector.memset(ones_f[:], 1.0)
            ones_c = cp.tile([1, 128], F16)
            nc.vector.memset(ones_c[:], 1.0)
            ident = cp.tile([128, 128], F16)
            make_identity(nc, ident[:])

            # ACT warmups: preload the Silu/Abs tables before the hot path.
            warm = acts.tile([128, 1], F32)
            nc.scalar.activation(
                warm[:], thr_sb[:], mybir.ActivationFunctionType.Silu
            )
            nc.scalar.activation(
                warm[:], thr_sb[:], mybir.ActivationFunctionType.Abs
            )
            nc.scalar.copy(warm[:], thr_sb[:])

            # resident weight tiles (fp8 streams are small enough to hold)
            wut_sb = acts.tile([128, FDC * D], FP8)
            wgt_sb = acts.tile([128, FDC * D], F16)
            wup_sb = acts.tile([128, NCD * FP], FP8)
            wdp_sb = acts.tile([128, NCF * DP], FP8)
            wdd_sb = acts.tile([128, NDVG * FSH], FP8)

            # activation scratch
            accg = acts.tile([128, FDC], F32)   # DVE-part gate accum
            accu = acts.tile([128, FDC], F32)   # DVE-part up accum
            dve_scr = acts.tile([128, D], F16)
            g_row = acts.tile([1, FP], F16)
            u_row = acts.tile([1, FP], F16)
            tg_col = acts.tile([128, NCF], F32)
            x1c = acts.tile([128, NCF], F32)
            abc = acts.tile([128, NCF], F32)
            mkc = acts.tile([128, NCF], F32)
            xmc = acts.tile([128, NCF], F32)
            u_col = acts.tile([128, NCF], F32)
            z_col = acts.tile([128, NCF], F16)
            z_row = acts.tile([1, FSH], F16)
            zrep = acts.tile([128, FSH], F16)
            osb = acts.tile([1, DP], F32)
            outd_sb = acts.tile([128, NDVG], F32)

            # ---- gate/up stream: all on the sync HWDGE ring, interleaved in
            # consumption order ----
            # (kind, matrix, offset_chunks, nchunks); matrix 0=up, 1=gate
            order = []
            toff_ = [0, 0]
            poff_ = [0, 0]
            for i, npc in enumerate(TPC):
                for mi in (0, 1):
                    order.append(("T", mi, toff_[mi], npc))
                    toff_[mi] += npc
                    order.append(("P", mi, poff_[mi], PPC[i]))
                    poff_[mi] += PPC[i]
            assert toff_ == [FDC, FDC] and poff_ == [NCD, NCD]

            gtiles = {}
            for kind, mi, off, npc in order:
                if kind == "T":
                    dram = wut_d if mi == 0 else wgt_d
                    sb = wut_sb if mi == 0 else wgt_sb
                    sl = slice(off * D, (off + npc) * D)
                    nc.sync.dma_start(out=sb[:, sl], in_=dram.ap()[:, sl])
                elif mi == 0:
                    sl = slice(off * FP, (off + npc) * FP)
                    nc.sync.dma_start(out=wup_sb[:, sl], in_=wup_d.ap()[:, sl])
                else:
                    t = gpool.tile([128, 8 * FP], F16, tag="gw", name="gw")
                    sl = slice(off * FP, (off + npc) * FP)
                    nc.sync.dma_start(
                        out=t[:, 0 : npc * FP], in_=wgp_d.ap()[:, sl]
                    )
                    gtiles[(mi, off)] = t

            # down stream follows on the same sync ring: HWDGE ring FIFO
            # pins its transfer order behind the gate/up bytes, no dummy
            # gating needed, and the ACT engine stays free for the z chain
            do = 0
            for npc in WDDPC:
                w = npc * FSH
                nc.sync.dma_start(
                    out=wdd_sb[:, do : do + w],
                    in_=wdd_d.ap()[:, do : do + w],
                )
                do += w
            po = 0
            for npc in WDPPC:
                w = npc * DP
                nc.sync.dma_start(
                    out=wdp_sb[:, po : po + w],
                    in_=wdp_d.ap()[:, po : po + w],
                )
                po += w

            with tc.tile_pool(name="ps1", bufs=1, space="PSUM") as ps1:
                pup = ps1.tile([1, 1024], F32)
                pgate = ps1.tile([1, 1024], F32)
                pzcu = ps1.tile([128, 16], F32)
                pzcg = ps1.tile([128, 16], F32)

                # consumption, in stream order: DVE affines for T pieces, PE
                # GEMVs for P pieces
                for kind, mi, off, npc in order:
                    if kind == "T":
                        sb = wut_sb if mi == 0 else wgt_sb
                        acct = accu if mi == 0 else accg
                        for j in range(npc):
                            c = off + j
                            nc.vector.affine_mul_reduce(
                                out=dve_scr[:, 0:D],
                                accum_out=acct[:, c : c + 1],
                                in0=sb[:, c * D : (c + 1) * D],
                                in1=xrep[:],
                                scale=1.0,
                                bias=0.0,
                            )
                    else:
                        accp = pup if mi == 0 else pgate
                        for cc in range(npc):
                            c = off + cc
                            if mi == 0:
                                rh = wup_sb[:, c * FP : (c + 1) * FP]
                            else:
                                t = gtiles[(mi, off)]
                                rh = t[:, cc * FP : (cc + 1) * FP]
                            for toff, tlen in FT:
                                nc.tensor.matmul(
                                    out=accp[0:1, toff : toff + tlen],
                                    lhsT=xcol[:, c : c + 1],
                                    rhs=rh[:, toff : toff + tlen],
                                    start=(c == 0),
                                    stop=(c == NCD - 1),
                                )

                # ---- u/g rows to column form; unified elementwise chain ----
                nc.scalar.copy(u_row[0:1, 0:FP], pup[0:1, 0:FP])
                nc.scalar.copy(g_row[0:1, 0:FP], pgate[0:1, 0:FP])
                for c in range(NCF - FDC):
                    pc = 128 if FDC + c < NCF - 1 else LASTF
                    nc.tensor.matmul(
                        out=pzcu[0:pc, c : c + 1],
                        lhsT=u_row[0:1, c * 128 : c * 128 + pc],
                        rhs=ones_f[0:1, 0:1],
                        start=True,
                        stop=True,
                    )
                    nc.tensor.matmul(
                        out=pzcg[0:pc, c : c + 1],
                        lhsT=g_row[0:1, c * 128 : c * 128 + pc],
                        rhs=ones_f[0:1, 0:1],
                        start=True,
                        stop=True,
                    )
                nc.scalar.copy(u_col[:, 0:FDC], accu[:, 0:FDC])
                nc.scalar.copy(u_col[:, FDC:NCF], pzcu[:, 0 : NCF - FDC])
                nc.scalar.copy(tg_col[:, 0:FDC], accg[:, 0:FDC])
                nc.scalar.copy(tg_col[:, FDC:NCF], pzcg[:, 0 : NCF - FDC])
                nc.scalar.activation(
                    x1c[:, 0:NCF], tg_col[:, 0:NCF],
                    mybir.ActivationFunctionType.Silu,
                )
                nc.scalar.activation(
                    abc[:, 0:NCF], x1c[:, 0:NCF],
                    mybir.ActivationFunctionType.Abs,
                )
                # mask carries the 1/WSCL^2 rescale of pup*(scaled wd)
                nc.vector.tensor_scalar(
                    out=mkc[:, 0:NCF], in0=abc[:, 0:NCF],
                    scalar1=thr_sb[:], scalar2=1.0 / (WSCL * WSCL),
                    op0=mybir.AluOpType.is_gt,
                    op1=mybir.AluOpType.mult,
                )
                nc.vector.tensor_mul(xmc[:, 0:NCF], x1c[:, 0:NCF], mkc[:, 0:NCF])
                nc.vector.tensor_mul(z_col[:, 0:NCF], u_col[:, 0:NCF], xmc[:, 0:NCF])

            with tc.tile_pool(name="ps2", bufs=1, space="PSUM") as ps2:
                pzrow = ps2.tile([1, 512], F32)
                przep = ps2.tile([128, 512], F32)
                pdp = ps2.tile([1, DP], F32)

                # z_row + zrep, slice-pipelined through one psum bank pair
                for soff, slen, c0, ncc in ZS:
                    for i in range(ncc):
                        c = c0 + i
                        pc = 128 if c < NCF - 1 else LASTF
                        nc.tensor.matmul(
                            out=pzrow[0:1, i * 128 : i * 128 + pc],
                            lhsT=z_col[0:pc, c : c + 1],
                            rhs=ident[0:pc, 0:pc],
                            start=True,
                            stop=True,
                        )
                    nc.scalar.copy(
                        z_row[0:1, soff : soff + slen], pzrow[0:1, 0:slen]
                    )
                    nc.tensor.matmul(
                        out=przep[:, 0:slen],
                        lhsT=ones_c[0:1, 0:128],
                        rhs=z_row[0:1, soff : soff + slen],
                        start=True,
                        stop=True,
                    )
                    nc.vector.tensor_copy(
                        zrep[:, soff : soff + slen], przep[:, 0:slen]
                    )

                # DVE down part first in program order so its affines are
                # not queued behind the osb copies (which wait on PE-down)
                for g in range(NDVG):
                    nc.vector.affine_mul_reduce(
                        out=dve_scr[:, 0:FSH],
                        accum_out=outd_sb[:, g : g + 1],
                        in0=wdd_sb[:, g * FSH : (g + 1) * FSH],
                        in1=zrep[:],
                        scale=1.0,
                        bias=0.0,
                    )

                # PE down part; ACT copies out tile-wise as chunks retire
                for c in range(NCF):
                    pc = 128 if c < NCF - 1 else LASTF
                    last = c == NCF - 1
                    for ti, (toff, tlen) in enumerate(DT):
                        nc.tensor.matmul(
                            out=pdp[0:1, toff : toff + tlen],
                            lhsT=z_col[0:pc, c : c + 1],
                            rhs=wdp_sb[0:pc, c * DP + toff : c * DP + toff + tlen],
                            start=(c == 0),
                            stop=last,
                        )
                        if last:
                            sl = slice(toff, toff + tlen)
                            nc.scalar.copy(osb[0:1, sl], pdp[0:1, sl])

            nc.sync.dma_start(out=outp_d.ap(), in_=osb[:])
            nc.sync.dma_start(out=outd_d.ap(), in_=outd_sb[:])

    nc.compile()
    return nc


def _get_nc():
    if "nc" not in _CACHE:
        _CACHE["nc"] = _build_nc()
    return _CACHE["nc"]


def _q8(W):
    return np.clip(
        np.asarray(W, dtype=np.float32) * WSCL, -15.5, 15.5
    ).astype(NF8)


def make_in_maps(x, Wup, Wgatet, Wdownt, threshold):
    """Shard full inputs into the 8 per-core input maps."""
    x_flat = np.asarray(x, dtype=np.float32).reshape(D)
    xcol = np.ascontiguousarray(x_flat.reshape(NCD, 128).T).astype(NF16)
    xf = np.ascontiguousarray(
        np.concatenate(
            [np.broadcast_to(x_flat.astype(NF16), (128, D)), xcol], axis=1
        )
    )
    thr = np.asarray(threshold, dtype=np.float32).reshape(1)
    Wup = np.asarray(Wup, dtype=np.float32)
    Wgatet = np.asarray(Wgatet, dtype=np.float32)
    Wdownt = np.asarray(Wdownt, dtype=np.float32)
    in_maps = []
    for i in range(NCORES):
        sl = slice(i * FSH, (i + 1) * FSH)
        wg_slice = Wgatet[:, sl]                  # [D, FSH] d-major
        wu_slice = Wup[sl, :]                     # [FSH, D] f-major
        wd_slice = Wdownt[sl, :]                  # [FSH, D] f-major

        # DVE layouts: f-major [128f, D] per chunk for f-rows [0:FD)
        wgT = np.ascontiguousarray(wg_slice.T)    # [FSH, D] f-major
        wgt = (
            wgT[:FD].reshape(FDC, 128, D).transpose(1, 0, 2)
            .reshape(128, FDC * D).astype(NF16)
        )
        wut = _q8(
            wu_slice[:FD].reshape(FDC, 128, D).transpose(1, 0, 2)
            .reshape(128, FDC * D)
        )
        # PE layouts: d-major [128d, FP] per chunk for f-rows [FD:FSH)
        wgp = (
            wg_slice[:, FD:].reshape(NCD, 128, FP).transpose(1, 0, 2)
            .reshape(128, NCD * FP).astype(NF16)
        )
        wuT = np.ascontiguousarray(wu_slice.T)    # [D, FSH] d-major
        wup = _q8(
            wuT[:, FD:].reshape(NCD, 128, FP).transpose(1, 0, 2)
            .reshape(128, NCD * FP)
        )
        wd_pad = np.zeros((FPAD, DP), dtype=np.float32)
        wd_pad[:FSH] = wd_slice[:, :DP]
        wdp = _q8(
            wd_pad.reshape(NCF, 128, DP).transpose(1, 0, 2).reshape(128, NCF * DP)
        )
        wdT = np.ascontiguousarray(wd_slice[:, DP:].T)  # [D-DP, FSH] d-major
        wdd = _q8(
            wdT.reshape(NDVG, 128, FSH).transpose(1, 0, 2).reshape(128, NDVG * FSH)
        )
        in_maps.append(
            {
                "xf": xf,
                "wut": np.ascontiguousarray(wut),
                "wgt": np.ascontiguousarray(wgt),
                "wup": np.ascontiguousarray(wup),
                "wgp": np.ascontiguousarray(wgp),
                "wdp": np.ascontiguousarray(wdp),
                "wdd": np.ascontiguousarray(wdd),
                "thr": thr,
            }
        )
    return in_maps


def run_sharded(x, Wup, Wgatet, Wdownt, threshold, trace=False, tmpdir=None):
    """Run on the 8 NeuronCores; returns (full_output, BassKernelResults)."""
    nc = _get_nc()
    in_maps = make_in_maps(x, Wup, Wgatet, Wdownt, threshold)
    res = run_bass_kernel_spmd(
        nc, in_maps, list(range(NCORES)), trace=trace, tmpdir=tmpdir
    )
    acc = np.zeros(D, dtype=np.float64)
    for r in res.results:
        acc[:DP] += r["outp"].reshape(DP).astype(np.float64)
        acc[DP:] += r["outd"].T.reshape(D - DP).astype(np.float64)
    out = acc.astype(np.float32).reshape(1, 1, D)
    return out, res


def kernel(x, Wup, Wgatet, Wdownt, threshold):
    out, _ = run_sharded(x, Wup, Wgatet, Wdownt, threshold)
    return out


# revision 24
# speedup vs baseline: 1.3042x; 1.1017x over previous
# CATS-SwiGLU decode kernel for TRN2 (8 NeuronCores, SPMD tensor-parallel).
# v7: mixed-precision weight streaming, dual-engine consumption.
#   gate fp16 (flag flips near |x1|>thr are the error wall), up/down fp8-e3m4
#   scaled by 128 (cancelled via the mask constant 1/2^14).
# Measured engine rates: PE GEMV (moving weights) ~N/2GHz+120ns per matmul
# (~170 Gelem/s), DVE affine_mul_reduce ~121 Gelem/s.  Neither alone can keep
# up with the ~378 GB/s HWDGE stream, so each matrix is split: f-chunks
# [0:640) reduce on the DVE over f-major [128,4096] tiles (accumulator lands
# f-on-partitions, pre-transposed for z), f-rows [640:1376) on the PE as
# d-major GEMVs; the down projection splits d-cols 2432 (PE) / 1664 (DVE).
# All gate/up/x DMA rides the sync HWDGE ring in consumption order (SWDGE is
# ~3x slower at descriptor generation - never use gpsimd rings for bulk);
# the down stream rides the scalar ring, pinned behind the last gate piece
# by the dummy-DMA WAW trick.
import sys

for _p in ("/opt/trn_rl_repo",):
    if _p not in sys.path:
        sys.path.insert(0, _p)

import numpy as np
import ml_dtypes

import concourse.bass as bass
import concourse.tile as tile
from concourse import bacc, mybir
from concourse.bass_utils import run_bass_kernel_spmd
from concourse.masks import make_identity

D = 4096
FF = 11008
NCORES = 8
FSH = FF // NCORES            # 1376 f-rows per core
NCD = D // 128                # 32 d-chunks (contraction for gate/up)
NCF = (FSH + 127) // 128      # 11 f-chunks
LASTF = FSH - 128 * (NCF - 1)  # 96 rows in the last f chunk

FDC = 4                       # f-chunks on the DVE for gate/up
FD = FDC * 128                # 640
FP = FSH - FD                 # 736 PE-part f width
FT = ((0, 512), (512, FP - 512))  # (512, 352)
FPAD = NCF * 128              # 1408 padded f-rows for wdp

DP = 2944                     # d-cols on PE for down
NDVG = (D - DP) // 128        # 9 DVE down groups
DT = ((0, 512), (512, 512), (1024, 512), (1536, 512), (2048, 512), (2560, 384))
ZS = ((0, 512, 0, 4), (512, 512, 4, 4), (1024, 352, 8, 3))  # z_row slices

WSCL = 128.0                  # e3m4 scale; 1/WSCL^2 folded into the mask
TPC = (1, 1, 1, 1)            # T (DVE-layout) piece sizes in chunks
PPC = (8, 8, 8, 8)            # P (PE-layout) piece sizes in d-chunks
GBUFS = 3                     # gate P-piece pool depth
WDDPC = (4, 3, 2)             # wdd pieces, in d-groups
WDPPC = (3, 3, 3, 2)          # wdp pieces, in f-chunks

F32 = mybir.dt.float32
F16 = mybir.dt.float16
FP8 = mybir.dt.float8e3
NF16 = np.float16
NF8 = ml_dtypes.float8_e3m4

_CACHE = {}


def _bcast(ap, parts):
    return bass.AP(tensor=ap.tensor, offset=ap.offset, ap=[[0, parts]] + list(ap.ap))


def _build_nc():
    nc = bacc.Bacc("TRN2", target_bir_lowering=False, debug=False)

    xf_d = nc.dram_tensor("xf", [128, D + NCD], F16, kind="ExternalInput")
    wut_d = nc.dram_tensor("wut", [128, FDC * D], FP8, kind="ExternalInput")
    wgt_d = nc.dram_tensor("wgt", [128, FDC * D], F16, kind="ExternalInput")
    wup_d = nc.dram_tensor("wup", [128, NCD * FP], FP8, kind="ExternalInput")
    wgp_d = nc.dram_tensor("wgp", [128, NCD * FP], F16, kind="ExternalInput")
    wdp_d = nc.dram_tensor("wdp", [128, NCF * DP], FP8, kind="ExternalInput")
    wdd_d = nc.dram_tensor("wdd", [128, NDVG * FSH], FP8, kind="ExternalInput")
    thr_d = nc.dram_tensor("thr", [1], F32, kind="ExternalInput")
    outp_d = nc.dram_tensor("outp", [DP], F32, kind="ExternalOutput")
    outd_d = nc.dram_tensor("outd", [128, NDVG], F32, kind="ExternalOutput")

    with tile.TileContext(nc) as tc:
        with (
            tc.tile_pool(name="const", bufs=1) as cp,
            tc.tile_pool(name="gpool", bufs=GBUFS) as gpool,
            tc.tile_pool(name="acts", bufs=1) as acts,
        ):
            # x (replicated row + column-chunked) is the first sync transfer;
            # thr rides the scalar ring.
            xf = cp.tile([128, D + NCD], F16)
            nc.sync.dma_start(out=xf[:], in_=xf_d.ap())
            xrep = xf[:, 0:D]
            xcol = xf[:, D : D + NCD]
            thr_sb = cp.tile([128, 1], F32)
            nc.scalar.dma_start(out=thr_sb[:], in_=_bcast(thr_d.ap(), 128))
            ones_f = cp.tile([1, 1], F16)
            nc.vector.memset(ones_f[:], 1.0)
            ones_c = cp.tile([1, 128], F16)
            nc.vector.memset(ones_c[:], 1.0)
            ident = cp.tile([128, 128], F16)
            make_identity(nc, ident[:])

            # ACT warmups: preload the Silu/Abs tables before the hot path.
            warm = acts.tile([128, 1], F32)
            nc.scalar.activation(
                warm[:], thr_sb[:], mybir.ActivationFunctionType.Silu
            )
            nc.scalar.activation(
                warm[:], thr_sb[:], mybir.ActivationFunctionType.Abs
            )
            nc.scalar.copy(warm[:], thr_sb[:])

            # resident weight tiles (fp8 streams are small enough to hold)
            wut_sb = acts.tile([128, FDC * D], FP8)
            wgt_sb = acts.tile([128, FDC * D], F16)
            wup_sb = acts.tile([128, NCD * FP], FP8)
            wdp_sb = acts.tile([128, NCF * DP], FP8)
            wdd_sb = acts.tile([128, NDVG * FSH], FP8)

            # activation scratch
            accg = acts.tile([128, FDC], F32)   # DVE-part gate accum
            accu = acts.tile([128, FDC], F32)   # DVE-part up accum
            dve_scr = acts.tile([128, D], F16)
            g_row = acts.tile([1, FP], F16)
            u_row = acts.tile([1, FP], F16)
            tg_col = acts.tile([128, NCF], F32)
            x1c = acts.tile([128, NCF], F32)
            abc = acts.tile([128, NCF], F32)
            mkc = acts.tile([128, NCF], F32)
            xmc = acts.tile([128, NCF], F32)
            u_col = acts.tile([128, NCF], F32)
            z_col = acts.tile([128, NCF], F16)
            z_row = acts.tile([1, FSH], F16)
            zrep = acts.tile([128, FSH], F16)
            osb = acts.tile([1, DP], F32)
            outd_sb = acts.tile([128, NDVG], F32)

            # ---- gate/up stream: all on the sync HWDGE ring, interleaved in
            # consumption order ----
            # (kind, matrix, offset_chunks, nchunks); matrix 0=up, 1=gate
            order = []
            toff_ = [0, 0]
            poff_ = [0, 0]
            last = len(TPC) - 1
            for i, npc in enumerate(TPC):
                if i < last:
                    for mi in (0, 1):
                        order.append(("T", mi, toff_[mi], npc))
                        toff_[mi] += npc
                        order.append(("P", mi, poff_[mi], PPC[i]))
                        poff_[mi] += PPC[i]
                else:
                    # final round: T pieces first so the DVE's last gate
                    # chunk lands well before the gate P stream ends
                    for mi in (0, 1):
                        order.append(("T", mi, toff_[mi], npc))
                        toff_[mi] += npc
                    for mi in (0, 1):
                        order.append(("P", mi, poff_[mi], PPC[i]))
                        poff_[mi] += PPC[i]
            assert toff_ == [FDC, FDC] and poff_ == [NCD, NCD]

            gtiles = {}
            for kind, mi, off, npc in order:
                if kind == "T":
                    dram = wut_d if mi == 0 else wgt_d
                    sb = wut_sb if mi == 0 else wgt_sb
                    sl = slice(off * D, (off + npc) * D)
                    nc.sync.dma_start(out=sb[:, sl], in_=dram.ap()[:, sl])
                elif mi == 0:
                    sl = slice(off * FP, (off + npc) * FP)
                    nc.sync.dma_start(out=wup_sb[:, sl], in_=wup_d.ap()[:, sl])
                else:
                    t = gpool.tile([128, 8 * FP], F16, tag="gw", name="gw")
                    sl = slice(off * FP, (off + npc) * FP)
                    nc.sync.dma_start(
                        out=t[:, 0 : npc * FP], in_=wgp_d.ap()[:, sl]
                    )
                    gtiles[(mi, off)] = t

            # down stream follows on the same sync ring: HWDGE ring FIFO
            # pins its transfer order behind the gate/up bytes, no dummy
            # gating needed, and the ACT engine stays free for the z chain
            do = 0
            for npc in WDDPC:
                w = npc * FSH
                nc.sync.dma_start(
                    out=wdd_sb[:, do : do + w],
                    in_=wdd_d.ap()[:, do : do + w],
                )
                do += w
            po = 0
            for npc in WDPPC:
                w = npc * DP
                nc.sync.dma_start(
                    out=wdp_sb[:, po : po + w],
                    in_=wdp_d.ap()[:, po : po + w],
                )
                po += w

            with tc.tile_pool(name="ps1", bufs=1, space="PSUM") as ps1:
                pup = ps1.tile([1, 1024], F32)
                pgate = ps1.tile([1, 1024], F32)
                pzcu = ps1.tile([128, 16], F32)
                pzcg = ps1.tile([128, 16], F32)

                # consumption, in stream order: DVE affines for T pieces, PE
                # GEMVs for P pieces
                for kind, mi, off, npc in order:
                    if kind == "T":
                        sb = wut_sb if mi == 0 else wgt_sb
                        acct = accu if mi == 0 else accg
                        for j in range(npc):
                            c = off + j
                            nc.vector.affine_mul_reduce(
                                out=dve_scr[:, 0:D],
                                accum_out=acct[:, c : c + 1],
                                in0=sb[:, c * D : (c + 1) * D],
                                in1=xrep[:],
                                scale=1.0,
                                bias=0.0,
                            )
                    else:
                        accp = pup if mi == 0 else pgate
                        for cc in range(npc):
                            c = off + cc
                            if mi == 0:
                                rh = wup_sb[:, c * FP : (c + 1) * FP]
                            else:
                                t = gtiles[(mi, off)]
                                rh = t[:, cc * FP : (cc + 1) * FP]
                            for toff, tlen in FT:
                                nc.tensor.matmul(
                                    out=accp[0:1, toff : toff + tlen],
                                    lhsT=xcol[:, c : c + 1],
                                    rhs=rh[:, toff : toff + tlen],
                                    start=(c == 0),
                                    stop=(c == NCD - 1),
                                )

                # ---- u/g rows to column form; unified elementwise chain ----
                nc.scalar.copy(u_row[0:1, 0:FP], pup[0:1, 0:FP])
                nc.scalar.copy(g_row[0:1, 0:FP], pgate[0:1, 0:FP])
                for c in range(NCF - FDC):
                    pc = 128 if FDC + c < NCF - 1 else LASTF
                    nc.tensor.matmul(
                        out=pzcu[0:pc, c : c + 1],
                        lhsT=u_row[0:1, c * 128 : c * 128 + pc],
                        rhs=ones_f[0:1, 0:1],
                        start=True,
                        stop=True,
                    )
                    nc.tensor.matmul(
                        out=pzcg[0:pc, c : c + 1],
                        lhsT=g_row[0:1, c * 128 : c * 128 + pc],
                        rhs=ones_f[0:1, 0:1],
                        start=True,
                        stop=True,
                    )
                nc.scalar.copy(u_col[:, 0:FDC], accu[:, 0:FDC])
                nc.scalar.copy(u_col[:, FDC:NCF], pzcu[:, 0 : NCF - FDC])
                nc.scalar.copy(tg_col[:, 0:FDC], accg[:, 0:FDC])
                nc.scalar.copy(tg_col[:, FDC:NCF], pzcg[:, 0 : NCF - FDC])
                nc.scalar.activation(
                    x1c[:, 0:NCF], tg_col[:, 0:NCF],
                    mybir.ActivationFunctionType.Silu,
                )
                nc.scalar.activation(
                    abc[:, 0:NCF], x1c[:, 0:NCF],
                    mybir.ActivationFunctionType.Abs,
                )
                # mask carries the 1/WSCL^2 rescale of pup*(scaled wd)
                nc.vector.tensor_scalar(
                    out=mkc[:, 0:NCF], in0=abc[:, 0:NCF],
                    scalar1=thr_sb[:], scalar2=1.0 / (WSCL * WSCL),
                    op0=mybir.AluOpType.is_gt,
                    op1=mybir.AluOpType.mult,
                )
                nc.vector.tensor_mul(xmc[:, 0:NCF], x1c[:, 0:NCF], mkc[:, 0:NCF])
                nc.vector.tensor_mul(z_col[:, 0:NCF], u_col[:, 0:NCF], xmc[:, 0:NCF])

            with tc.tile_pool(name="ps2", bufs=1, space="PSUM") as ps2:
                pzrow = ps2.tile([1, 512], F32)
                przep = ps2.tile([128, 512], F32)
                pdp = ps2.tile([1, DP], F32)

                # z_row + zrep, slice-pipelined through one psum bank pair
                for soff, slen, c0, ncc in ZS:
                    for i in range(ncc):
                        c = c0 + i
                        pc = 128 if c < NCF - 1 else LASTF
                        nc.tensor.matmul(
                            out=pzrow[0:1, i * 128 : i * 128 + pc],
                            lhsT=z_col[0:pc, c : c + 1],
                            rhs=ident[0:pc, 0:pc],
                            start=True,
                            stop=True,
                        )
                    nc.scalar.copy(
                        z_row[0:1, soff : soff + slen], pzrow[0:1, 0:slen]
                    )
                    nc.tensor.matmul(
                        out=przep[:, 0:slen],
                        lhsT=ones_c[0:1, 0:128],
                        rhs=z_row[0:1, soff : soff + slen],
                        start=True,
                        stop=True,
                    )
                    nc.vector.tensor_copy(
                        zrep[:, soff : soff + slen], przep[:, 0:slen]
                    )

                # DVE down part first in program order so its affines are
                # not queued behind the osb copies (which wait on PE-down)
                for g in range(NDVG):
                    nc.vector.affine_mul_reduce(
                        out=dve_scr[:, 0:FSH],
                        accum_out=outd_sb[:, g : g + 1],
                        in0=wdd_sb[:, g * FSH : (g + 1) * FSH],
                        in1=zrep[:],
                        scale=1.0,
                        bias=0.0,
                    )

                # PE down part; ACT copies out tile-wise as chunks retire
                for c in range(NCF):
                    pc = 128 if c < NCF - 1 else LASTF
                    last = c == NCF - 1
                    for ti, (toff, tlen) in enumerate(DT):
                        nc.tensor.matmul(
                            out=pdp[0:1, toff : toff + tlen],
                            lhsT=z_col[0:pc, c : c + 1],
                            rhs=wdp_sb[0:pc, c * DP + toff : c * DP + toff + tlen],
                            start=(c == 0),
                            stop=last,
                        )
                        if last:
                            sl = slice(toff, toff + tlen)
                            nc.scalar.copy(osb[0:1, sl], pdp[0:1, sl])

            nc.sync.dma_start(out=outp_d.ap(), in_=osb[:])
            nc.sync.dma_start(out=outd_d.ap(), in_=outd_sb[:])

    nc.compile()
    return nc


def _get_nc():
    if "nc" not in _CACHE:
        _CACHE["nc"] = _build_nc()
    return _CACHE["nc"]


def _q8(W):
    return np.clip(
        np.asarray(W, dtype=np.float32) * WSCL, -15.5, 15.5
    ).astype(NF8)


def make_in_maps(x, Wup, Wgatet, Wdownt, threshold):
    """Shard full inputs into the 8 per-core input maps."""
    x_flat = np.asarray(x, dtype=np.float32).reshape(D)
    xcol = np.ascontiguousarray(x_flat.reshape(NCD, 128).T).astype(NF16)
    xf = np.ascontiguousarray(
        np.concatenate(
            [np.broadcast_to(x_flat.astype(NF16), (128, D)), xcol], axis=1
        )
    )
    thr = np.asarray(threshold, dtype=np.float32).reshape(1)
    Wup = np.asarray(Wup, dtype=np.float32)
    Wgatet = np.asarray(Wgatet, dtype=np.float32)
    Wdownt = np.asarray(Wdownt, dtype=np.float32)
    in_maps = []
    for i in range(NCORES):
        sl = slice(i * FSH, (i + 1) * FSH)
        wg_slice = Wgatet[:, sl]                  # [D, FSH] d-major
        wu_slice = Wup[sl, :]                     # [FSH, D] f-major
        wd_slice = Wdownt[sl, :]                  # [FSH, D] f-major

        # DVE layouts: f-major [128f, D] per chunk for f-rows [0:FD)
        wgT = np.ascontiguousarray(wg_slice.T)    # [FSH, D] f-major
        wgt = (
            wgT[:FD].reshape(FDC, 128, D).transpose(1, 0, 2)
            .reshape(128, FDC * D).astype(NF16)
        )
        wut = _q8(
            wu_slice[:FD].reshape(FDC, 128, D).transpose(1, 0, 2)
            .reshape(128, FDC * D)
        )
        # PE layouts: d-major [128d, FP] per chunk for f-rows [FD:FSH)
        wgp = (
            wg_slice[:, FD:].reshape(NCD, 128, FP).transpose(1, 0, 2)
            .reshape(128, NCD * FP).astype(NF16)
        )
        wuT = np.ascontiguousarray(wu_slice.T)    # [D, FSH] d-major
        wup = _q8(
            wuT[:, FD:].reshape(NCD, 128, FP).transpose(1, 0, 2)
            .reshape(128, NCD * FP)
        )
        wd_pad = np.zeros((FPAD, DP), dtype=np.float32)
        wd_pad[:FSH] = wd_slice[:, :DP]
        wdp = _q8(
            wd_pad.reshape(NCF, 128, DP).transpose(1, 0, 2).reshape(128, NCF * DP)
        )
        wdT = np.ascontiguousarray(wd_slice[:, DP:].T)  # [D-DP, FSH] d-major
        wdd = _q8(
            wdT.reshape(NDVG, 128, FSH).transpose(1, 0, 2).reshape(128, NDVG * FSH)
        )
        in_maps.append(
            {
                "xf": xf,
                "wut": np.ascontiguousarray(wut),
                "wgt": np.ascontiguousarray(wgt),
                "wup": np.ascontiguousarray(wup),
                "wgp": np.ascontiguousarray(wgp),
                "wdp": np.ascontiguousarray(wdp),
                "wdd": np.ascontiguousarray(wdd),
                "thr": thr,
            }
        )
    return in_maps


def run_sharded(x, Wup, Wgatet, Wdownt, threshold, trace=False, tmpdir=None):
    """Run on the 8 NeuronCores; returns (full_output, BassKernelResults)."""
    nc = _get_nc()
    in_maps = make_in_maps(x, Wup, Wgatet, Wdownt, threshold)
    res = run_bass_kernel_spmd(
        nc, in_maps, list(range(NCORES)), trace=trace, tmpdir=tmpdir
    )
    acc = np.zeros(D, dtype=np.float64)
    for r in res.results:
        acc[:DP] += r["outp"].reshape(DP).astype(np.float64)
        acc[DP:] += r["outd"].T.reshape(D - DP).astype(np.float64)
    out = acc.astype(np.float32).reshape(1, 1, D)
    return out, res


def kernel(x, Wup, Wgatet, Wdownt, threshold):
    out, _ = run_sharded(x, Wup, Wgatet, Wdownt, threshold)
    return out
